# revision 5
# baseline (speedup 1.0000x reference)
# InternLM2-7B decode-step paged attention on 8 Trainium2 NeuronCores, v4.
#
# Sharding (tensor-parallel, per the source hooks):
#   - wqkv column-sharded: core c gets q heads 4c..4c+3 and kv head c
#   - wo row-sharded: core c gets rows for q heads 4c..4c+3
#   - KV cache sharded along the kv-head dim: core c gets head c
#   - output projection partials summed on the host (the all-reduce)
#
# v4 (vs v2): HBM traffic cut via int8 transport of the KV cache.
#   - V cache fully int8, dequantized on the Scalar engine (activation Copy
#     with a per-partition f32 scale vector = per-position scale).
#   - K cache: even chunk-groups int8 (cast on Vector+GpSimd engines as raw
#     ints), odd groups bf16. Dequant scale folded into the exp() activation
#     scale vector (per-partition = per-position in the S^T layout).
#   - V phase octo-packed: stationary = attnT[:, c, 32o:32o+32] (8 seqs x 4
#     heads), moving = 4 seqs' V rows [l, 512] -> out [32, 512] at psum
#     partition base 32o (tile_position; quadrant 96 verified on HW). Two
#     psum banks (A: moving seqs j%8<4, B: j%8>=4); garbage blocks ignored
#     at extraction. N=512 moving beats the v2 per-seq N=4 matmuls.
#   - normalization via per-partition reciprocal vector at extraction
#     (psum row = 4s+h), no replicate matmul.
#   - wo loaded during the main phase (gpsimd ring) instead of up front.
import os
import sys

for _p in (
    "/opt/trn_rl_repo",
    "/root/.axon_site",
    "/root/.axon_site/_ro/trn_rl_repo",
    "/root/.axon_site/_ro/pypackages",
):
    if os.path.isdir(_p) and _p not in sys.path:
        sys.path.append(_p)

import numpy as np
import ml_dtypes

BF16NP = ml_dtypes.bfloat16

import concourse.bass as bass
from concourse import bacc
import concourse.mybir as mybir
import concourse.tile as tile
from concourse.masks import make_identity

B = 32          # batch (decoding sequences)
H = 32          # query heads
KVH = 8         # kv heads
G = 4           # query heads per kv head (= per core)
HD = 128        # head dim
D = 4096        # model dim
W = (G + 2) * HD  # per-core qkv shard width = 768
L = 4096        # kv positions per sequence
NCH = L // 128  # 32 l-chunks of 128
CGK = 2         # l-chunks per kT DMA tile / psum slab
CGN = NCH // CGK  # 16 chunk groups
KT_ = D // 128  # 32 contraction tiles for the qkv projection
BLOCK = 64
NBLK = 64
NCORES = 8
THETA = 1e6
R = G * B       # 128 row-cols (s-major: col = 4*s + h)

F32 = mybir.dt.float32
BF16 = mybir.dt.bfloat16
I8 = mybir.dt.int8
SCALE = 1.0 / float(np.sqrt(HD))

# group processing order: bf16 (odd) first, then its int8 (even) partner, so
# the PE has direct-DMA work while the first casts run.
GORDER = []
for _g in range(0, CGN, 2):
    GORDER.append(_g + 1)
    GORDER.append(_g)
# DVE/GPS split of the int8 K cast along the seq dim (DVE hits the 2x_2p
# fast path ~154Gelem/s; GPSIMD is ~33Gelem/s)
KSPL = 28


def _emit(nc, tc, hT, wq, qsc, wo, kTb, kTi, vti, mz, bK, cV, cs, y):
    import contextlib

    Exp = mybir.ActivationFunctionType.Exp
    Copy = mybir.ActivationFunctionType.Copy

    with contextlib.ExitStack() as ctx:
        singles = ctx.enter_context(tc.tile_pool(name="singles", bufs=1))
        wqp = ctx.enter_context(tc.tile_pool(name="wqp", bufs=2))
        ktp8 = ctx.enter_context(tc.tile_pool(name="ktp8", bufs=2))
        ktbp = ctx.enter_context(tc.tile_pool(name="ktbp", bufs=3))
        vt8p = ctx.enter_context(tc.tile_pool(name="vt8p", bufs=2))
        vtbp = ctx.enter_context(tc.tile_pool(name="vtbp", bufs=3))
        attp_pool = ctx.enter_context(tc.tile_pool(name="attp", bufs=4))
        stg = ctx.enter_context(tc.tile_pool(name="stg", bufs=3))
        # PSUM (8 banks): psA 3 + psV 2 + psD 3
        psA = ctx.enter_context(tc.tile_pool(name="psA", bufs=3, space="PSUM"))
        psV = ctx.enter_context(tc.tile_pool(name="psV", bufs=1, space="PSUM"))
        psD = ctx.enter_context(tc.tile_pool(name="psD", bufs=1, space="PSUM"))

        ident = singles.tile([128, 128], F32)
        make_identity(nc, ident)

        # ---- input loads
        hT_sb = singles.tile([128, KT_, B], BF16)
        nc.sync.dma_start(hT_sb, hT)
        cs_sb = singles.tile([B, HD], F32)
        nc.sync.dma_start(cs_sb, cs)
        mz_sb = singles.tile([128, NCH, B], BF16)
        nc.scalar.dma_start(mz_sb, mz)
        bK_sb = singles.tile([128, CGN], F32)
        nc.sync.dma_start(bK_sb, bK)
        cV_sb = singles.tile([128, CGN], F32)
        nc.sync.dma_start(cV_sb, cV)

        qT_buf = singles.tile([128, B, G], BF16)
        k_newT = singles.tile([128, B], F32)
        tmp_kq = singles.tile([128, B, G], F32)
        ones_bf = singles.tile([128, 1], BF16)
        nc.vector.memset(ones_bf, 1.0)
        ones_f = singles.tile([128, 1], F32)
        nc.vector.memset(ones_f, 1.0)
        pnew_row = singles.tile([1, R], BF16)
        vnew_row = singles.tile([1, B, HD], BF16)
        sums_f = singles.tile([1, R], F32)
        rc_row = singles.tile([1, R], F32)
        rc_col = singles.tile([128, 1], F32)
        outT_bf = singles.tile([128, B, G], BF16)
        wo_sb = singles.tile([128, G, D], BF16)

        # ---- fused QKV projection: qkv[B, W] = hT.T @ (wq_int8 -> bf16),
        # dequant via per-column scales folded in after the psum copy ----
        qsc_sb = singles.tile([B, W], F32)
        nc.sync.dma_start(qsc_sb, qsc)
        ps_q0 = psA.tile([128, 512], F32, tag="scp")
        ps_q1 = psA.tile([128, 512], F32, tag="scp")
        for tq in range(KT_ // 2):
            wt8 = wqp.tile([128, 2, W], I8, tag="wt8")
            nc.sync.dma_start(wt8, wq[:, tq * 2 : (tq + 1) * 2, :])
            wt = wqp.tile([128, 2, W], BF16, tag="wt")
            nc.scalar.copy(wt.rearrange("p u w -> p (u w)"),
                           wt8.rearrange("p u w -> p (u w)"))
            for u in range(2):
                t = tq * 2 + u
                nc.tensor.matmul(ps_q0[:B, 0:384], lhsT=hT_sb[:, t, :],
                                 rhs=wt[:, u, 0:384],
                                 start=(t == 0), stop=(t == KT_ - 1))
                nc.tensor.matmul(ps_q1[:B, 0:384], lhsT=hT_sb[:, t, :],
                                 rhs=wt[:, u, 384:W],
                                 start=(t == 0), stop=(t == KT_ - 1))
        qkv_sb = singles.tile([B, W], F32)
        nc.vector.tensor_mul(qkv_sb[:, 0:384], ps_q0[:B, 0:384],
                             qsc_sb[:, 0:384])
        nc.vector.tensor_mul(qkv_sb[:, 384:W], ps_q1[:B, 0:384],
                             qsc_sb[:, 384:W])

        # ---- RoPE on q (G heads) and k (1 head); v passthrough ----
        q_sb = singles.tile([B, G * HD], F32)
        k_sb = singles.tile([B, HD], F32)
        v_sb = singles.tile([B, HD], F32)
        nc.vector.tensor_copy(v_sb, qkv_sb[:, (G + 1) * HD : (G + 2) * HD])
        cosv = cs_sb[:, 0:64]
        sinv = cs_sb[:, 64:128]
        for j in range(G + 1):
            src = qkv_sb[:, j * HD : (j + 1) * HD]
            dst = q_sb[:, j * HD : (j + 1) * HD] if j < G else k_sb[:, :]
            a = src[:, 0:64]
            b = src[:, 64:128]
            t1 = stg.tile([B, 64], F32, tag="rt1")
            t2 = stg.tile([B, 64], F32, tag="rt2")
            nc.vector.tensor_mul(t1, a, cosv)
            nc.vector.tensor_mul(t2, b, sinv)
            nc.vector.tensor_sub(dst[:, 0:64], t1, t2)
            t3 = stg.tile([B, 64], F32, tag="rt1")
            t4 = stg.tile([B, 64], F32, tag="rt2")
            nc.vector.tensor_mul(t3, b, cosv)
            nc.vector.tensor_mul(t4, a, sinv)
            nc.vector.tensor_add(dst[:, 64:128], t3, t4)

        # ---- qT (pre-scaled, bf16): qT_buf[d, s, h]; k_newT[d, s] ----
        for h in range(G):
            ps_t = psA.tile([128, 512], F32, tag="scp")
            nc.tensor.transpose(ps_t[:, :B], q_sb[:, h * HD : (h + 1) * HD],
                                ident[:B, :B])
            nc.vector.tensor_scalar_mul(out=qT_buf[:, :, h], in0=ps_t[:, :B],
                                        scalar1=SCALE)
        ps_t = psA.tile([128, 512], F32, tag="scp")
        nc.tensor.transpose(ps_t[:, :B], k_sb[:, :], ident[:B, :B])
        nc.vector.tensor_copy(k_newT, ps_t[:, :B])

        # ---- new-token staging (heavy DVE/ACT work deferred to the tail) --
        v_sbb = singles.tile([B, HD], BF16)
        nc.vector.tensor_copy(v_sbb, v_sb)
        nc.gpsimd.dma_start(vnew_row[0:1, :, :], v_sbb[:, :])
        # wo load on the gpsimd ring: overlaps the main phase
        nc.gpsimd.dma_start(wo_sb, wo.rearrange("(h p) n -> p h n", p=128))
        psM = psD.tile([1, 512], F32, tag="misc")

        # ---- V-phase psum banks: [128, 512] x2, rows 4s+h, col (s%4)*128+d
        psVA = psV.tile([128, 512], F32, tag="va")
        psVB = psV.tile([128, 512], F32, tag="vb")

        # ---- main loop over chunk groups (bf16/int8 alternating); the
        # V/sums matmuls for group g are emitted during group g+1 so the PE
        # never head-of-line blocks on the exp->mask chain.
        nproc = 0

        def emit_v(attp, vtb):
            nonlocal nproc
            for u in range(CGK):
                nproc += 1
                nc.tensor.matmul(psM[0:1, 0:R], lhsT=ones_bf[:, 0:1],
                                 rhs=attp[:, u, :],
                                 start=(nproc == 1), stop=(nproc == NCH))
                for o in range(4):
                    lt = attp[:, u, 32 * o : 32 * o + 32]
                    kw = {"tile_position": (0, 96)} if o == 3 else {}
                    nc.tensor.matmul(psVA[32 * o : 32 * o + 32, :], lhsT=lt,
                                     rhs=vtb[:, u, 8 * o : 8 * o + 4, :],
                                     start=(nproc == 1), stop=False, **kw)
                    nc.tensor.matmul(psVB[32 * o : 32 * o + 32, :], lhsT=lt,
                                     rhs=vtb[:, u, 8 * o + 4 : 8 * o + 8, :],
                                     start=(nproc == 1), stop=False, **kw)

        prev = None
        for gi, cg in enumerate(GORDER):
            ktb_t = ktbp.tile([128, B, CGK, 128], BF16, tag="kt")
            if cg % 2 == 0:
                kt8 = ktp8.tile([128, B, CGK, 128], I8, tag="kt8")
                nc.sync.dma_start(kt8, kTi[cg // 2, :, :, :, :])
                nc.vector.tensor_copy(ktb_t[:, 0:KSPL, :, :],
                                      kt8[:, 0:KSPL, :, :])
                nc.gpsimd.tensor_copy(ktb_t[:, KSPL:B, :, :],
                                      kt8[:, KSPL:B, :, :])
            else:
                nc.sync.dma_start(ktb_t, kTb[cg // 2, :, :, :, :])
            # V pair tile: one DMA + one fused cast per chunk-group
            vt8 = vt8p.tile([128, CGK, B, HD], I8, tag="vt8")
            nc.sync.dma_start(vt8, vti[cg, :, :, :, :])
            scp = psA.tile([128, 512], F32, tag="scp")
            for s in range(B):
                for u in range(CGK):
                    o = u * 128 + 4 * s
                    nc.tensor.matmul(scp[:, o : o + 4], lhsT=ktb_t[:, s, u, :],
                                     rhs=qT_buf[:, s, :],
                                     start=True, stop=True)
            attp = attp_pool.tile([128, CGK, R], BF16, tag="at")
            nc.scalar.activation(out=attp.rearrange("p u r -> p (u r)"),
                                 in_=scp[:, 0 : CGK * 128],
                                 func=Exp, scale=bK_sb[:, cg : cg + 1])
            attp4 = attp.rearrange("p u (s h) -> p u s h", h=G)
            nc.vector.tensor_mul(
                attp4, attp4,
                mz_sb[:, CGK * cg : CGK * cg + CGK, :, None].broadcast_to(
                    [128, CGK, B, G]))
            vtb = vtbp.tile([128, CGK, B, HD], BF16, tag="vtb")
            if cg % 3 != 0:
                nc.scalar.activation(
                    out=vtb.rearrange("p u s d -> p (u s d)"),
                    in_=vt8.rearrange("p u s d -> p (u s d)"),
                    func=Copy, scale=cV_sb[:, cg : cg + 1])
            else:
                nc.vector.tensor_scalar_mul(
                    out=vtb.rearrange("p u s d -> p (u s d)"),
                    in0=vt8.rearrange("p u s d -> p (u s d)"),
                    scalar1=cV_sb[:, cg : cg + 1])
            if prev is not None:
                emit_v(*prev)
            prev = (attp, vtb)
            if gi == 6:
                # new-token p_new = exp(qT . k_new), off the critical tail
                nc.vector.tensor_mul(
                    tmp_kq, qT_buf,
                    k_newT[:, :, None].broadcast_to([128, B, G]))
                ps_pn = psA.tile([128, 512], F32, tag="scp")
                nc.tensor.matmul(ps_pn[0:1, 0:R], lhsT=ones_f[:, 0:1],
                                 rhs=tmp_kq[:, :, :], start=True, stop=True)
                nc.scalar.activation(out=pnew_row[0:1, :],
                                     in_=ps_pn[0:1, 0:R], func=Exp)
        emit_v(*prev)

        # ---- new-token append to the V accumulation (rank-1) ----
        for o in range(4):
            lt = pnew_row[0:1, 32 * o : 32 * o + 32]
            kw = {"tile_position": (0, 96)} if o == 3 else {}
            nc.tensor.matmul(psVA[32 * o : 32 * o + 32, :], lhsT=lt,
                             rhs=vnew_row[0:1, 8 * o : 8 * o + 4, :],
                             start=False, stop=True, **kw)
            nc.tensor.matmul(psVB[32 * o : 32 * o + 32, :], lhsT=lt,
                             rhs=vnew_row[0:1, 8 * o + 4 : 8 * o + 8, :],
                             start=False, stop=True, **kw)

        # ---- denominators: rc_col[4s+h] = 1/(sums + p_new) ----
        nc.vector.tensor_add(sums_f, psM[0:1, 0:R], pnew_row[0:1, :])
        nc.vector.reciprocal(rc_row, sums_f)
        psR = psD.tile([128, 1], F32, tag="rct")
        nc.tensor.transpose(psR[:, 0:1], rc_row[0:1, :], ident[0:1, 0:1])
        nc.vector.tensor_copy(rc_col, psR[:, 0:1])

        # ---- normalize full banks to SBUF (partition-aligned ops) ----
        sbA = singles.tile([128, 512], F32)
        sbB = singles.tile([128, 512], F32)
        nc.vector.tensor_scalar_mul(out=sbA, in0=psVA[:, :], scalar1=rc_col)
        nc.vector.tensor_scalar_mul(out=sbB, in0=psVB[:, :], scalar1=rc_col)

        # ---- outT[d, 4s+h]: transpose each 128-col block, then gather the
        # valid columns (free-dim strided AP) into outT_bf ----
        outT_fl = outT_bf.rearrange("p s h -> p (s h)")
        for bi, sb in enumerate((sbA, sbB)):
            for m in range(4):
                psO = psD.tile([128, 128], F32, tag="ot")
                nc.tensor.transpose(psO[:, :], sb[:, 128 * m : 128 * m + 128],
                                    ident[:, :])
                # valid cols: 4s+h for s%4==m, s%8 in {m or m+4} -> cols
                # {32k + 4*(m + 4*bi) + h, k=0..3, h=0..3}
                base = 4 * (m + 4 * bi)
                src = psO.rearrange("p (k r) -> p k r", k=4)[:, :, base : base + 4]
                dst = outT_fl.rearrange("p (k r) -> p k r", k=4)[:, :, base : base + 4]
                nc.vector.tensor_copy(dst, src)

        # ---- output projection partial: y = outT.T @ wo_shard ----
        for n in range(D // 512):
            ps_y = psA.tile([128, 512], F32, tag="scp")
            for h in range(G):
                nc.tensor.matmul(ps_y[:B, :], lhsT=outT_bf[:, :, h],
                                 rhs=wo_sb[:, h, n * 512 : (n + 1) * 512],
                                 start=(h == 0), stop=(h == G - 1))
            yst = stg.tile([B, 512], F32, tag="yst")
            nc.any.tensor_copy(yst, ps_y[:B, :])
            nc.scalar.dma_start(y[:, n * 512 : (n + 1) * 512], yst)


_NC_CACHE = None


def build_bass():
    global _NC_CACHE
    if _NC_CACHE is not None:
        return _NC_CACHE
    nc = bacc.Bacc("TRN2")
    hT = nc.dram_tensor("hT", [128, KT_, B], BF16, kind="ExternalInput")
    wq = nc.dram_tensor("wq", [128, KT_, W], I8, kind="ExternalInput")
    qsc = nc.dram_tensor("qsc", [B, W], F32, kind="ExternalInput")
    wo = nc.dram_tensor("wo", [G * HD, D], BF16, kind="ExternalInput")
    kTb = nc.dram_tensor("kTb", [CGN // 2, 128, B, CGK, 128], BF16,
                         kind="ExternalInput")
    kTi = nc.dram_tensor("kTi", [CGN // 2, 128, B, CGK, 128], I8,
                         kind="ExternalInput")
    vti = nc.dram_tensor("vti", [CGN, 128, CGK, B, HD], I8,
                         kind="ExternalInput")
    mz = nc.dram_tensor("mz", [128, NCH, B], BF16, kind="ExternalInput")
    bK = nc.dram_tensor("bK", [128, CGN], F32, kind="ExternalInput")
    cV = nc.dram_tensor("cV", [128, CGN], F32, kind="ExternalInput")
    cs = nc.dram_tensor("cs", [B, HD], F32, kind="ExternalInput")
    y = nc.dram_tensor("y", [B, D], F32, kind="ExternalOutput")
    with tile.TileContext(nc) as tc:
        _emit(nc, tc, hT[:, :, :], wq[:, :, :], qsc[:, :], wo[:, :],
              kTb[:, :, :, :, :], kTi[:, :, :, :, :], vti[:, :, :, :, :],
              mz[:, :, :], bK[:, :], cV[:, :], cs[:, :], y[:, :])
    nc.finalize()
    _NC_CACHE = nc
    return nc


def make_host_inputs(hidden_states, wqkv, wo, k_cache, v_cache,
                     position_ids_1d, block_offsets, kv_seqlens):
    """Shard + preprocess full inputs into 8 per-core in_maps."""
    hidden_states = np.asarray(hidden_states, dtype=np.float32)
    wqkv = np.asarray(wqkv, dtype=np.float32)
    wo = np.asarray(wo, dtype=np.float32)
    k_cache = np.asarray(k_cache, dtype=np.float32)
    v_cache = np.asarray(v_cache, dtype=np.float32)
    position_ids_1d = np.asarray(position_ids_1d, dtype=np.int32)
    block_offsets = np.asarray(block_offsets, dtype=np.int32)
    kv_seqlens = np.asarray(kv_seqlens, dtype=np.int32)

    hTd = np.ascontiguousarray(
        hidden_states.T.reshape(KT_, 128, B).transpose(1, 0, 2)
    ).astype(BF16NP)  # [128, KT_, B]

    inv_freq = (1.0 / (THETA ** (np.arange(0, HD, 2, dtype=np.float64) / HD)))
    ang = position_ids_1d.astype(np.float64)[:, None] * inv_freq[None, :]
    cs_host = np.concatenate(
        [np.cos(ang), np.sin(ang)], axis=1).astype(np.float32)  # [B, 128]

    # validity: cache position j valid iff j < seqlen-1
    j = np.arange(L, dtype=np.int64)[None, :]
    valid = (j < (kv_seqlens.astype(np.int64)[:, None] - 1))  # [B, L] bool
    validT = valid.reshape(B, NCH, 128).transpose(2, 1, 0)  # [p, c, s]
    mz_host = np.ascontiguousarray(validT.astype(np.float32)).astype(BF16NP)

    ident_blocks = np.array_equal(block_offsets.ravel(),
                                  np.arange(B * NBLK, dtype=np.int64))

    kx = np.moveaxis(k_cache, 2, 0)  # [KVH, NUM_BLOCKS, BLOCK, HD]
    vx = np.moveaxis(v_cache, 2, 0)

    in_maps = []
    for c in range(NCORES):
        if ident_blocks:
            kg = kx[c].reshape(B, L, HD)
            vg = vx[c].reshape(B, L, HD)
        else:
            kg = kx[c][block_offsets].reshape(B, L, HD)
            vg = vx[c][block_offsets].reshape(B, L, HD)

        # K: per-position scale shared across seqs; even chunk-groups int8
        kabs = np.abs(kg).max(axis=(0, 2))  # [L]
        bscale = (kabs / 127.0).astype(np.float32)
        bscale = np.maximum(bscale, 1e-20)
        # bK[p, cgroup]: exp-scale for psum partition p of chunk c; both
        # chunks of a group share the DMA tile; scale indexed per chunk ->
        # use per-chunk-group layout [128, CGN] with chunk u offset folded:
        # NOTE the exp call uses bK[:, cg] for BOTH chunks of group cg, so
        # the scale must be equal for chunk 2cg and 2cg+1 at each partition.
        # Make it so: quantize with a per-(p, group) scale (max over the two
        # chunks' positions at that partition).
        bs2 = bscale.reshape(NCH, 128)  # [c, p]
        bgrp = np.maximum(bs2[0::2, :], bs2[1::2, :])  # [CGN, p]
        bgrp[1::2, :] = 1.0  # odd groups stay bf16: exp scale 1
        bK_host = np.ascontiguousarray(bgrp.T).astype(np.float32)  # [p, CGN]
        bfull = np.repeat(bgrp, 2, axis=0).reshape(L)  # [L] effective scale
        k_int = np.clip(np.round(kg / bfull[None, :, None]), -127, 127
                        ).astype(np.int8)
        kall = kg.reshape(B, CGN, CGK, 128, HD).transpose(1, 4, 0, 2, 3)
        kTb_c = np.ascontiguousarray(kall[1::2]).astype(BF16NP)
        kTi_c = np.ascontiguousarray(
            k_int.reshape(B, CGN, CGK, 128, HD).transpose(1, 4, 0, 2, 3)[0::2])

        # V: per-(p, chunk-group) scale shared across seqs, fully int8
        vabs = np.abs(vg).max(axis=(0, 2))  # [L]
        vs2 = (vabs / 127.0).reshape(NCH, 128)  # [c, p]
        vgrp = np.maximum(np.maximum(vs2[0::2, :], vs2[1::2, :]), 1e-20)
        cV_host = np.ascontiguousarray(vgrp.T).astype(np.float32)  # [p, CGN]
        vfull = np.repeat(vgrp, 2, axis=0).reshape(L)
        v_int = np.clip(np.round(vg / vfull[None, :, None]), -127, 127
                        ).astype(np.int8)
        # vti[cg, p(l), u, s, d]
        vti_c = np.ascontiguousarray(
            v_int.reshape(B, CGN, CGK, 128, HD).transpose(1, 3, 2, 0, 4))

        wq_full = np.concatenate([
            wqkv[:, c * G * HD : (c + 1) * G * HD],
            wqkv[:, H * HD + c * HD : H * HD + (c + 1) * HD],
            wqkv[:, (H + KVH) * HD + c * HD : (H + KVH) * HD + (c + 1) * HD],
        ], axis=1)  # [D, W]
        wsc = np.maximum(np.abs(wq_full).max(axis=0) / 127.0, 1e-20)  # [W]
        wq_int = np.clip(np.round(wq_full / wsc[None, :]), -127, 127
                         ).astype(np.int8)
        wq_c = np.ascontiguousarray(
            wq_int.reshape(KT_, 128, W).transpose(1, 0, 2))
        qsc_c = np.ascontiguousarray(
            np.broadcast_to(wsc[None, :].astype(np.float32), (B, W)))
        wo_c = np.ascontiguousarray(
            wo[c * G * HD : (c + 1) * G * HD, :]).astype(BF16NP)  # [G*HD, D]
        in_maps.append(dict(hT=hTd, wq=wq_c, qsc=qsc_c, wo=wo_c, kTb=kTb_c, kTi=kTi_c,
                            vti=vti_c, mz=mz_host, bK=bK_host, cV=cV_host,
                            cs=cs_host))
    return in_maps


def kernel(**inputs):
    from concourse.bass_utils import run_bass_kernel_spmd

    in_maps = make_host_inputs(
        inputs["hidden_states"], inputs["wqkv"], inputs["wo"],
        inputs["k_cache"], inputs["v_cache"], inputs["position_ids_1d"],
        inputs["block_offsets"], inputs["kv_seqlens"])
    nc = build_bass()
    res = run_bass_kernel_spmd(nc, in_maps, core_ids=list(range(NCORES)))
    y = np.zeros((B, D), dtype=np.float32)
    for r in res.results:
        y += np.asarray(r["y"], dtype=np.float32)
    return y


# revision 6
# speedup vs baseline: 1.0276x; 1.0276x over previous
# InternLM2-7B decode-step paged attention on 8 Trainium2 NeuronCores, v4.
#
# Sharding (tensor-parallel, per the source hooks):
#   - wqkv column-sharded: core c gets q heads 4c..4c+3 and kv head c
#   - wo row-sharded: core c gets rows for q heads 4c..4c+3
#   - KV cache sharded along the kv-head dim: core c gets head c
#   - output projection partials summed on the host (the all-reduce)
#
# v4 (vs v2): HBM traffic cut via int8 transport of the KV cache.
#   - V cache fully int8, dequantized on the Scalar engine (activation Copy
#     with a per-partition f32 scale vector = per-position scale).
#   - K cache: even chunk-groups int8 (cast on Vector+GpSimd engines as raw
#     ints), odd groups bf16. Dequant scale folded into the exp() activation
#     scale vector (per-partition = per-position in the S^T layout).
#   - V phase octo-packed: stationary = attnT[:, c, 32o:32o+32] (8 seqs x 4
#     heads), moving = 4 seqs' V rows [l, 512] -> out [32, 512] at psum
#     partition base 32o (tile_position; quadrant 96 verified on HW). Two
#     psum banks (A: moving seqs j%8<4, B: j%8>=4); garbage blocks ignored
#     at extraction. N=512 moving beats the v2 per-seq N=4 matmuls.
#   - normalization via per-partition reciprocal vector at extraction
#     (psum row = 4s+h), no replicate matmul.
#   - wo loaded during the main phase (gpsimd ring) instead of up front.
import os
import sys

for _p in (
    "/opt/trn_rl_repo",
    "/root/.axon_site",
    "/root/.axon_site/_ro/trn_rl_repo",
    "/root/.axon_site/_ro/pypackages",
):
    if os.path.isdir(_p) and _p not in sys.path:
        sys.path.append(_p)

import numpy as np
import ml_dtypes

BF16NP = ml_dtypes.bfloat16

import concourse.bass as bass
from concourse import bacc
import concourse.mybir as mybir
import concourse.tile as tile
from concourse.masks import make_identity

B = 32          # batch (decoding sequences)
H = 32          # query heads
KVH = 8         # kv heads
G = 4           # query heads per kv head (= per core)
HD = 128        # head dim
D = 4096        # model dim
W = (G + 2) * HD  # per-core qkv shard width = 768
L = 4096        # kv positions per sequence
NCH = L // 128  # 32 l-chunks of 128
CGK = 2         # l-chunks per kT DMA tile / psum slab
CGN = NCH // CGK  # 16 chunk groups
KT_ = D // 128  # 32 contraction tiles for the qkv projection
BLOCK = 64
NBLK = 64
NCORES = 8
THETA = 1e6
R = G * B       # 128 row-cols (s-major: col = 4*s + h)

F32 = mybir.dt.float32
BF16 = mybir.dt.bfloat16
I8 = mybir.dt.int8
SCALE = 1.0 / float(np.sqrt(HD))

# group processing order: bf16 (odd) first, then its int8 (even) partner, so
# the PE has direct-DMA work while the first casts run.
GORDER = []
for _g in range(0, CGN, 2):
    GORDER.append(_g + 1)
    GORDER.append(_g)
# DVE/GPS split of the int8 K cast along the seq dim (DVE hits the 2x_2p
# fast path ~154Gelem/s; GPSIMD is ~33Gelem/s)
KSPL = 28


def _emit(nc, tc, hT, wq, wo, kTb, kTi, vti, mz, bK, cV, cs, y):
    import contextlib

    Exp = mybir.ActivationFunctionType.Exp
    Copy = mybir.ActivationFunctionType.Copy

    with contextlib.ExitStack() as ctx:
        singles = ctx.enter_context(tc.tile_pool(name="singles", bufs=1))
        wqp = ctx.enter_context(tc.tile_pool(name="wqp", bufs=2))
        ktp8 = ctx.enter_context(tc.tile_pool(name="ktp8", bufs=2))
        ktbp = ctx.enter_context(tc.tile_pool(name="ktbp", bufs=3))
        vt8p = ctx.enter_context(tc.tile_pool(name="vt8p", bufs=2))
        vtbp = ctx.enter_context(tc.tile_pool(name="vtbp", bufs=3))
        attp_pool = ctx.enter_context(tc.tile_pool(name="attp", bufs=4))
        stg = ctx.enter_context(tc.tile_pool(name="stg", bufs=3))
        # PSUM (8 banks): psA 3 + psV 2 + psD 3
        psA = ctx.enter_context(tc.tile_pool(name="psA", bufs=3, space="PSUM"))
        psV = ctx.enter_context(tc.tile_pool(name="psV", bufs=1, space="PSUM"))
        psD = ctx.enter_context(tc.tile_pool(name="psD", bufs=1, space="PSUM"))

        ident = singles.tile([128, 128], F32)
        make_identity(nc, ident)

        # ---- input loads
        hT_sb = singles.tile([128, KT_, B], BF16)
        nc.sync.dma_start(hT_sb, hT)
        cs_sb = singles.tile([B, HD], F32)
        nc.sync.dma_start(cs_sb, cs)
        mz_sb = singles.tile([128, NCH, B], BF16)
        nc.scalar.dma_start(mz_sb, mz)
        bK_sb = singles.tile([128, CGN], F32)
        nc.sync.dma_start(bK_sb, bK)
        cV_sb = singles.tile([128, CGN], F32)
        nc.sync.dma_start(cV_sb, cV)

        qT_buf = singles.tile([128, B, G], BF16)
        k_newT = singles.tile([128, B], F32)
        tmp_kq = singles.tile([128, B, G], F32)
        ones_bf = singles.tile([128, 1], BF16)
        nc.vector.memset(ones_bf, 1.0)
        ones_f = singles.tile([128, 1], F32)
        nc.vector.memset(ones_f, 1.0)
        pnew_row = singles.tile([1, R], BF16)
        vnew_row = singles.tile([1, B, HD], BF16)
        sums_f = singles.tile([1, R], F32)
        rc_row = singles.tile([1, R], F32)
        rc_col = singles.tile([128, 1], F32)
        outT_bf = singles.tile([128, B, G], BF16)
        wo_sb = singles.tile([128, G, D], BF16)

        # ---- fused QKV projection: qkv[B, W] = hT.T @ wq ----
        ps_q0 = psA.tile([128, 512], F32, tag="scp")
        ps_q1 = psA.tile([128, 512], F32, tag="scp")
        for tq in range(KT_ // 2):
            wt = wqp.tile([128, 2, W], BF16, tag="wt")
            nc.sync.dma_start(wt, wq[:, tq * 2 : (tq + 1) * 2, :])
            for u in range(2):
                t = tq * 2 + u
                nc.tensor.matmul(ps_q0[:B, 0:384], lhsT=hT_sb[:, t, :],
                                 rhs=wt[:, u, 0:384],
                                 start=(t == 0), stop=(t == KT_ - 1))
                nc.tensor.matmul(ps_q1[:B, 0:384], lhsT=hT_sb[:, t, :],
                                 rhs=wt[:, u, 384:W],
                                 start=(t == 0), stop=(t == KT_ - 1))
        qkv_sb = singles.tile([B, W], F32)
        nc.vector.tensor_copy(qkv_sb[:, 0:384], ps_q0[:B, 0:384])
        nc.vector.tensor_copy(qkv_sb[:, 384:W], ps_q1[:B, 0:384])

        # ---- RoPE on q (G heads) and k (1 head); v passthrough ----
        q_sb = singles.tile([B, G * HD], F32)
        k_sb = singles.tile([B, HD], F32)
        v_sb = singles.tile([B, HD], F32)
        nc.vector.tensor_copy(v_sb, qkv_sb[:, (G + 1) * HD : (G + 2) * HD])
        cosv = cs_sb[:, 0:64]
        sinv = cs_sb[:, 64:128]
        for j in range(G + 1):
            src = qkv_sb[:, j * HD : (j + 1) * HD]
            dst = q_sb[:, j * HD : (j + 1) * HD] if j < G else k_sb[:, :]
            a = src[:, 0:64]
            b = src[:, 64:128]
            t1 = stg.tile([B, 64], F32, tag="rt1")
            t2 = stg.tile([B, 64], F32, tag="rt2")
            nc.vector.tensor_mul(t1, a, cosv)
            nc.vector.tensor_mul(t2, b, sinv)
            nc.vector.tensor_sub(dst[:, 0:64], t1, t2)
            t3 = stg.tile([B, 64], F32, tag="rt1")
            t4 = stg.tile([B, 64], F32, tag="rt2")
            nc.vector.tensor_mul(t3, b, cosv)
            nc.vector.tensor_mul(t4, a, sinv)
            nc.vector.tensor_add(dst[:, 64:128], t3, t4)

        # ---- qT (pre-scaled, bf16): qT_buf[d, s, h]; k_newT[d, s] ----
        for h in range(G):
            ps_t = psA.tile([128, 512], F32, tag="scp")
            nc.tensor.transpose(ps_t[:, :B], q_sb[:, h * HD : (h + 1) * HD],
                                ident[:B, :B])
            nc.vector.tensor_scalar_mul(out=qT_buf[:, :, h], in0=ps_t[:, :B],
                                        scalar1=SCALE)
        ps_t = psA.tile([128, 512], F32, tag="scp")
        nc.tensor.transpose(ps_t[:, :B], k_sb[:, :], ident[:B, :B])
        nc.vector.tensor_copy(k_newT, ps_t[:, :B])

        # ---- new-token staging (heavy DVE/ACT work deferred to the tail) --
        v_sbb = singles.tile([B, HD], BF16)
        nc.vector.tensor_copy(v_sbb, v_sb)
        nc.gpsimd.dma_start(vnew_row[0:1, :, :], v_sbb[:, :])
        # wo load on the gpsimd ring: overlaps the main phase
        nc.gpsimd.dma_start(wo_sb, wo.rearrange("(h p) n -> p h n", p=128))
        psM = psD.tile([1, 512], F32, tag="misc")

        # ---- V-phase psum banks: [128, 512] x2, rows 4s+h, col (s%4)*128+d
        psVA = psV.tile([128, 512], F32, tag="va")
        psVB = psV.tile([128, 512], F32, tag="vb")

        # ---- main loop over chunk groups (bf16/int8 alternating); the
        # V/sums matmuls for group g are emitted during group g+1 so the PE
        # never head-of-line blocks on the exp->mask chain.
        nproc = 0

        def emit_v(attp, vtb):
            nonlocal nproc
            for u in range(CGK):
                nproc += 1
                nc.tensor.matmul(psM[0:1, 0:R], lhsT=ones_bf[:, 0:1],
                                 rhs=attp[:, u, :],
                                 start=(nproc == 1), stop=(nproc == NCH))
                for o in range(4):
                    lt = attp[:, u, 32 * o : 32 * o + 32]
                    kw = {"tile_position": (0, 96)} if o == 3 else {}
                    nc.tensor.matmul(psVA[32 * o : 32 * o + 32, :], lhsT=lt,
                                     rhs=vtb[:, u, 8 * o : 8 * o + 4, :],
                                     start=(nproc == 1), stop=False, **kw)
                    nc.tensor.matmul(psVB[32 * o : 32 * o + 32, :], lhsT=lt,
                                     rhs=vtb[:, u, 8 * o + 4 : 8 * o + 8, :],
                                     start=(nproc == 1), stop=False, **kw)

        prev = None
        for gi, cg in enumerate(GORDER):
            ktb_t = ktbp.tile([128, B, CGK, 128], BF16, tag="kt")
            if cg % 2 == 0:
                kt8 = ktp8.tile([128, B, CGK, 128], I8, tag="kt8")
                nc.sync.dma_start(kt8, kTi[cg // 2, :, :, :, :])
                nc.vector.tensor_copy(ktb_t[:, 0:KSPL, :, :],
                                      kt8[:, 0:KSPL, :, :])
                nc.gpsimd.tensor_copy(ktb_t[:, KSPL:B, :, :],
                                      kt8[:, KSPL:B, :, :])
            else:
                nc.sync.dma_start(ktb_t, kTb[cg // 2, :, :, :, :])
            # V pair tile: one DMA + one fused cast per chunk-group
            vt8 = vt8p.tile([128, CGK, B, HD], I8, tag="vt8")
            nc.sync.dma_start(vt8, vti[cg, :, :, :, :])
            scp = psA.tile([128, 512], F32, tag="scp")
            for s in range(B):
                for u in range(CGK):
                    o = u * 128 + 4 * s
                    nc.tensor.matmul(scp[:, o : o + 4], lhsT=ktb_t[:, s, u, :],
                                     rhs=qT_buf[:, s, :],
                                     start=True, stop=True)
            attp = attp_pool.tile([128, CGK, R], BF16, tag="at")
            nc.scalar.activation(out=attp.rearrange("p u r -> p (u r)"),
                                 in_=scp[:, 0 : CGK * 128],
                                 func=Exp, scale=bK_sb[:, cg : cg + 1])
            attp4 = attp.rearrange("p u (s h) -> p u s h", h=G)
            nc.vector.tensor_mul(
                attp4, attp4,
                mz_sb[:, CGK * cg : CGK * cg + CGK, :, None].broadcast_to(
                    [128, CGK, B, G]))
            vtb = vtbp.tile([128, CGK, B, HD], BF16, tag="vtb")
            if cg % 3 != 0:
                nc.scalar.activation(
                    out=vtb.rearrange("p u s d -> p (u s d)"),
                    in_=vt8.rearrange("p u s d -> p (u s d)"),
                    func=Copy, scale=cV_sb[:, cg : cg + 1])
            else:
                nc.vector.tensor_scalar_mul(
                    out=vtb.rearrange("p u s d -> p (u s d)"),
                    in0=vt8.rearrange("p u s d -> p (u s d)"),
                    scalar1=cV_sb[:, cg : cg + 1])
            if prev is not None:
                emit_v(*prev)
            prev = (attp, vtb)
            if gi == 6:
                # new-token p_new = exp(qT . k_new), off the critical tail
                nc.vector.tensor_mul(
                    tmp_kq, qT_buf,
                    k_newT[:, :, None].broadcast_to([128, B, G]))
                ps_pn = psA.tile([128, 512], F32, tag="scp")
                nc.tensor.matmul(ps_pn[0:1, 0:R], lhsT=ones_f[:, 0:1],
                                 rhs=tmp_kq[:, :, :], start=True, stop=True)
                nc.scalar.activation(out=pnew_row[0:1, :],
                                     in_=ps_pn[0:1, 0:R], func=Exp)
        emit_v(*prev)

        # ---- new-token append to the V accumulation (rank-1) ----
        for o in range(4):
            lt = pnew_row[0:1, 32 * o : 32 * o + 32]
            kw = {"tile_position": (0, 96)} if o == 3 else {}
            nc.tensor.matmul(psVA[32 * o : 32 * o + 32, :], lhsT=lt,
                             rhs=vnew_row[0:1, 8 * o : 8 * o + 4, :],
                             start=False, stop=True, **kw)
            nc.tensor.matmul(psVB[32 * o : 32 * o + 32, :], lhsT=lt,
                             rhs=vnew_row[0:1, 8 * o + 4 : 8 * o + 8, :],
                             start=False, stop=True, **kw)

        # ---- denominators: rc_col[4s+h] = 1/(sums + p_new) ----
        nc.vector.tensor_add(sums_f, psM[0:1, 0:R], pnew_row[0:1, :])
        nc.vector.reciprocal(rc_row, sums_f)
        psR = psD.tile([128, 1], F32, tag="rct")
        nc.tensor.transpose(psR[:, 0:1], rc_row[0:1, :], ident[0:1, 0:1])
        nc.vector.tensor_copy(rc_col, psR[:, 0:1])

        # ---- normalize full banks to SBUF (partition-aligned ops) ----
        sbA = singles.tile([128, 512], F32)
        sbB = singles.tile([128, 512], F32)
        nc.vector.tensor_scalar_mul(out=sbA, in0=psVA[:, :], scalar1=rc_col)
        nc.vector.tensor_scalar_mul(out=sbB, in0=psVB[:, :], scalar1=rc_col)

        # ---- outT[d, 4s+h]: transpose each 128-col block, then gather the
        # valid columns (free-dim strided AP) into outT_bf ----
        outT_fl = outT_bf.rearrange("p s h -> p (s h)")
        for bi, sb in enumerate((sbA, sbB)):
            for m in range(4):
                psO = psD.tile([128, 128], F32, tag="ot")
                nc.tensor.transpose(psO[:, :], sb[:, 128 * m : 128 * m + 128],
                                    ident[:, :])
                # valid cols: 4s+h for s%4==m, s%8 in {m or m+4} -> cols
                # {32k + 4*(m + 4*bi) + h, k=0..3, h=0..3}
                base = 4 * (m + 4 * bi)
                src = psO.rearrange("p (k r) -> p k r", k=4)[:, :, base : base + 4]
                dst = outT_fl.rearrange("p (k r) -> p k r", k=4)[:, :, base : base + 4]
                nc.vector.tensor_copy(dst, src)

        # ---- output projection partial: y = outT.T @ wo_shard ----
        for n in range(D // 512):
            ps_y = psA.tile([128, 512], F32, tag="scp")
            for h in range(G):
                nc.tensor.matmul(ps_y[:B, :], lhsT=outT_bf[:, :, h],
                                 rhs=wo_sb[:, h, n * 512 : (n + 1) * 512],
                                 start=(h == 0), stop=(h == G - 1))
            yst = stg.tile([B, 512], F32, tag="yst")
            nc.any.tensor_copy(yst, ps_y[:B, :])
            nc.scalar.dma_start(y[:, n * 512 : (n + 1) * 512], yst)


_NC_CACHE = None


def build_bass():
    global _NC_CACHE
    if _NC_CACHE is not None:
        return _NC_CACHE
    nc = bacc.Bacc("TRN2")
    hT = nc.dram_tensor("hT", [128, KT_, B], BF16, kind="ExternalInput")
    wq = nc.dram_tensor("wq", [128, KT_, W], BF16, kind="ExternalInput")
    wo = nc.dram_tensor("wo", [G * HD, D], BF16, kind="ExternalInput")
    kTb = nc.dram_tensor("kTb", [CGN // 2, 128, B, CGK, 128], BF16,
                         kind="ExternalInput")
    kTi = nc.dram_tensor("kTi", [CGN // 2, 128, B, CGK, 128], I8,
                         kind="ExternalInput")
    vti = nc.dram_tensor("vti", [CGN, 128, CGK, B, HD], I8,
                         kind="ExternalInput")
    mz = nc.dram_tensor("mz", [128, NCH, B], BF16, kind="ExternalInput")
    bK = nc.dram_tensor("bK", [128, CGN], F32, kind="ExternalInput")
    cV = nc.dram_tensor("cV", [128, CGN], F32, kind="ExternalInput")
    cs = nc.dram_tensor("cs", [B, HD], F32, kind="ExternalInput")
    y = nc.dram_tensor("y", [B, D], F32, kind="ExternalOutput")
    with tile.TileContext(nc) as tc:
        _emit(nc, tc, hT[:, :, :], wq[:, :, :], wo[:, :],
              kTb[:, :, :, :, :], kTi[:, :, :, :, :], vti[:, :, :, :, :],
              mz[:, :, :], bK[:, :], cV[:, :], cs[:, :], y[:, :])
    nc.finalize()
    _NC_CACHE = nc
    return nc


def make_host_inputs(hidden_states, wqkv, wo, k_cache, v_cache,
                     position_ids_1d, block_offsets, kv_seqlens):
    """Shard + preprocess full inputs into 8 per-core in_maps."""
    hidden_states = np.asarray(hidden_states, dtype=np.float32)
    wqkv = np.asarray(wqkv, dtype=np.float32)
    wo = np.asarray(wo, dtype=np.float32)
    k_cache = np.asarray(k_cache, dtype=np.float32)
    v_cache = np.asarray(v_cache, dtype=np.float32)
    position_ids_1d = np.asarray(position_ids_1d, dtype=np.int32)
    block_offsets = np.asarray(block_offsets, dtype=np.int32)
    kv_seqlens = np.asarray(kv_seqlens, dtype=np.int32)

    hTd = np.ascontiguousarray(
        hidden_states.T.reshape(KT_, 128, B).transpose(1, 0, 2)
    ).astype(BF16NP)  # [128, KT_, B]

    inv_freq = (1.0 / (THETA ** (np.arange(0, HD, 2, dtype=np.float64) / HD)))
    ang = position_ids_1d.astype(np.float64)[:, None] * inv_freq[None, :]
    cs_host = np.concatenate(
        [np.cos(ang), np.sin(ang)], axis=1).astype(np.float32)  # [B, 128]

    # validity: cache position j valid iff j < seqlen-1
    j = np.arange(L, dtype=np.int64)[None, :]
    valid = (j < (kv_seqlens.astype(np.int64)[:, None] - 1))  # [B, L] bool
    validT = valid.reshape(B, NCH, 128).transpose(2, 1, 0)  # [p, c, s]
    mz_host = np.ascontiguousarray(validT.astype(np.float32)).astype(BF16NP)

    ident_blocks = np.array_equal(block_offsets.ravel(),
                                  np.arange(B * NBLK, dtype=np.int64))

    kx = np.moveaxis(k_cache, 2, 0)  # [KVH, NUM_BLOCKS, BLOCK, HD]
    vx = np.moveaxis(v_cache, 2, 0)

    in_maps = []
    for c in range(NCORES):
        if ident_blocks:
            kg = kx[c].reshape(B, L, HD)
            vg = vx[c].reshape(B, L, HD)
        else:
            kg = kx[c][block_offsets].reshape(B, L, HD)
            vg = vx[c][block_offsets].reshape(B, L, HD)

        # K: per-position scale shared across seqs; even chunk-groups int8
        kabs = np.abs(kg).max(axis=(0, 2))  # [L]
        bscale = (kabs / 127.0).astype(np.float32)
        bscale = np.maximum(bscale, 1e-20)
        # bK[p, cgroup]: exp-scale for psum partition p of chunk c; both
        # chunks of a group share the DMA tile; scale indexed per chunk ->
        # use per-chunk-group layout [128, CGN] with chunk u offset folded:
        # NOTE the exp call uses bK[:, cg] for BOTH chunks of group cg, so
        # the scale must be equal for chunk 2cg and 2cg+1 at each partition.
        # Make it so: quantize with a per-(p, group) scale (max over the two
        # chunks' positions at that partition).
        bs2 = bscale.reshape(NCH, 128)  # [c, p]
        bgrp = np.maximum(bs2[0::2, :], bs2[1::2, :])  # [CGN, p]
        bgrp[1::2, :] = 1.0  # odd groups stay bf16: exp scale 1
        bK_host = np.ascontiguousarray(bgrp.T).astype(np.float32)  # [p, CGN]
        bfull = np.repeat(bgrp, 2, axis=0).reshape(L)  # [L] effective scale
        k_int = np.clip(np.round(kg / bfull[None, :, None]), -127, 127
                        ).astype(np.int8)
        kall = kg.reshape(B, CGN, CGK, 128, HD).transpose(1, 4, 0, 2, 3)
        kTb_c = np.ascontiguousarray(kall[1::2]).astype(BF16NP)
        kTi_c = np.ascontiguousarray(
            k_int.reshape(B, CGN, CGK, 128, HD).transpose(1, 4, 0, 2, 3)[0::2])

        # V: per-(p, chunk-group) scale shared across seqs, fully int8
        vabs = np.abs(vg).max(axis=(0, 2))  # [L]
        vs2 = (vabs / 127.0).reshape(NCH, 128)  # [c, p]
        vgrp = np.maximum(np.maximum(vs2[0::2, :], vs2[1::2, :]), 1e-20)
        cV_host = np.ascontiguousarray(vgrp.T).astype(np.float32)  # [p, CGN]
        vfull = np.repeat(vgrp, 2, axis=0).reshape(L)
        v_int = np.clip(np.round(vg / vfull[None, :, None]), -127, 127
                        ).astype(np.int8)
        # vti[cg, p(l), u, s, d]
        vti_c = np.ascontiguousarray(
            v_int.reshape(B, CGN, CGK, 128, HD).transpose(1, 3, 2, 0, 4))

        wq_c = np.ascontiguousarray(np.concatenate([
            wqkv[:, c * G * HD : (c + 1) * G * HD],
            wqkv[:, H * HD + c * HD : H * HD + (c + 1) * HD],
            wqkv[:, (H + KVH) * HD + c * HD : (H + KVH) * HD + (c + 1) * HD],
        ], axis=1).reshape(KT_, 128, W).transpose(1, 0, 2)).astype(BF16NP)
        wo_c = np.ascontiguousarray(
            wo[c * G * HD : (c + 1) * G * HD, :]).astype(BF16NP)  # [G*HD, D]
        in_maps.append(dict(hT=hTd, wq=wq_c, wo=wo_c, kTb=kTb_c, kTi=kTi_c,
                            vti=vti_c, mz=mz_host, bK=bK_host, cV=cV_host,
                            cs=cs_host))
    return in_maps


def kernel(**inputs):
    from concourse.bass_utils import run_bass_kernel_spmd

    in_maps = make_host_inputs(
        inputs["hidden_states"], inputs["wqkv"], inputs["wo"],
        inputs["k_cache"], inputs["v_cache"], inputs["position_ids_1d"],
        inputs["block_offsets"], inputs["kv_seqlens"])
    nc = build_bass()
    res = run_bass_kernel_spmd(nc, in_maps, core_ids=list(range(NCORES)))
    y = np.zeros((B, D), dtype=np.float32)
    for r in res.results:
        y += np.asarray(r["y"], dtype=np.float32)
    return y
